# revision 9
# baseline (speedup 1.0000x reference)
"""Chamfer distance loss kernel for Trainium2 (8 NeuronCores, Bass/Tile).

Problem: x=(8,4096,128), y=(8,4096,128) fp32.
  pairwise[b,n,m] = ||x[b,n]-y[b,m]||
  loss = mean_{b,m}( min_n pairwise ) + mean_{n,m}( min_b pairwise )

Sharding: over M (the only axis surviving both reductions) -> fully
data-parallel, no collectives. Each core owns a 512-wide m-slice and
emits small partial-sum tensors; the host combines them.

On-device math (per core, m-slice of 512):
  s'[b,n,m] = (x2[b,n]-128) + (y2[b,m]-128) - 2*x.y   (shifted by -256 so
              fp16 SBUF copies stay precise; undone via sqrt(. + 256))
  PE   : K=128 matmul (-2xy, fp16 in / fp32 psum) + K=2 matmul adding the
         rank-1 terms x2s[n]*1[m] + 1[n]*y2s[m], accumulated in PSUM.
  ACT  : Identity copy PSUM->SBUF fp16 (FD=2048 groups of 4 batches).
  DVE  : fp16 2x-mode tensor_tensor mins:
           forward : running min into two (128,2048) quad accumulators
           backward: min-tree across the 8 batch slabs per n-tile
  ACT  : Sqrt(min + 256) with accum_out -> per-partition row sums.
  Tail : PE-transpose fwd accumulators, DVE reduce_min over n-lanes,
         add y2, Sqrt+accum -> one (128,1) forward partial per core.
"""

import numpy as np

B, N, M, D = 8, 4096, 4096, 128
NCORES = 8
MC = M // NCORES          # 512 m-columns per core
NT = N // 128             # 32 n-tiles
SHIFT = np.float64(D)     # 128.0 subtracted from each of x2, y2
ACC_INIT = 30000.0        # fp16 "+inf" for running mins

_cached = {}


def _build_nc():
    from contextlib import ExitStack

    import concourse.bass as bass
    import concourse.mybir as mybir
    import concourse.tile as tile
    from concourse.alu_op_type import AluOpType
    from concourse.masks import make_identity

    dt = mybir.dt
    AF = mybir.ActivationFunctionType
    AX = mybir.AxisListType

    nc = bass.Bass()

    xtp_d = nc.dram_tensor("xtp", [128, B * N], dt.float16, kind="ExternalInput")
    auglhs_d = nc.dram_tensor("auglhs", [2, B * N], dt.float16, kind="ExternalInput")
    yts_d = nc.dram_tensor("yts", [128, B * MC], dt.float16, kind="ExternalInput")
    augrhs_d = nc.dram_tensor("augrhs", [2, B * MC], dt.float16, kind="ExternalInput")
    y2arr_d = nc.dram_tensor("y2arr", [128, NT], dt.float32, kind="ExternalInput")
    bwd_d = nc.dram_tensor("bwd_sums", [128, NT], dt.float32, kind="ExternalOutput")
    fwd_d = nc.dram_tensor("fwd_psum", [128, 1], dt.float32, kind="ExternalOutput")

    with tile.TileContext(nc) as tc, ExitStack() as ctx:
        const_pool = ctx.enter_context(tc.tile_pool(name="const", bufs=1))
        xc_pool = ctx.enter_context(tc.tile_pool(name="xchunks", bufs=1))
        s_pool = ctx.enter_context(tc.tile_pool(name="s", bufs=4))
        t_pool = ctx.enter_context(tc.tile_pool(name="tree", bufs=2))
        acc_pool = ctx.enter_context(tc.tile_pool(name="acc", bufs=1))
        psum_pool = ctx.enter_context(tc.tile_pool(name="psum", bufs=2, space="PSUM"))

        # --- constants / inputs resident in SBUF ---
        yts_sb = const_pool.tile([128, B * MC], dt.float16, tag="yts")
        nc.sync.dma_start(yts_sb[:], yts_d[:, :])
        augrhs_sb = const_pool.tile([2, B * MC], dt.float16, tag="augrhs")
        nc.sync.dma_start(augrhs_sb[:], augrhs_d[:, :])
        auglhs_sb = const_pool.tile([2, B * N], dt.float16, tag="auglhs")
        nc.sync.dma_start(auglhs_sb[:], auglhs_d[:, :])
        y2arr_sb = const_pool.tile([128, NT], dt.float32, tag="y2arr")
        nc.sync.dma_start(y2arr_sb[:], y2arr_d[:, :])
        identity = const_pool.tile([128, 128], dt.float16, tag="ident")
        make_identity(nc, identity[:])
        bias256 = const_pool.tile([128, 1], dt.float32, tag="b256")
        nc.vector.memset(bias256[:], 256.0)

        # x^T chunks: 8 x (128, 4096) fp16; chunk ci covers n-tiles 4ci..4ci+3
        xch = []
        for ci in range(8):
            t = xc_pool.tile([128, 4096], dt.float16, tag=f"xc{ci}")
            nc.sync.dma_start(t[:], xtp_d[:, ci * 4096:(ci + 1) * 4096])
            xch.append(t)

        # --- persistent accumulators ---
        accA = acc_pool.tile([128, 2048], dt.float16, tag="accA")  # fwd mins b=0..3
        accB = acc_pool.tile([128, 2048], dt.float16, tag="accB")  # fwd mins b=4..7
        nc.vector.memset(accA[:], ACC_INIT)
        nc.vector.memset(accB[:], ACC_INIT)
        bwd_sb = acc_pool.tile([128, NT], dt.float32, tag="bwd_sb")
        fwd_mins = acc_pool.tile([128, NT], dt.float32, tag="fwd_mins")

        # --- main loop over n-tiles ---
        for nt_ in range(NT):
            ci = nt_ // 4
            s_tiles = []
            for g in range(2):
                psum_g = psum_pool.tile([128, 2048], dt.float32, tag="psum")
                for bl in range(4):
                    b = g * 4 + bl
                    col = nt_ * 1024 + b * 128
                    xoff = col - ci * 4096
                    slab = psum_g[:, bl * 512:(bl + 1) * 512]
                    nc.tensor.matmul(
                        slab,
                        xch[ci][:, xoff:xoff + 128],
                        yts_sb[:, b * MC:(b + 1) * MC],
                        start=True, stop=False,
                    )
                    nc.tensor.matmul(
                        slab,
                        auglhs_sb[:, col:col + 128],
                        augrhs_sb[:, b * MC:(b + 1) * MC],
                        start=False, stop=True,
                    )
                s_g = s_pool.tile([128, 2048], dt.float16, tag="s")
                nc.scalar.activation(s_g[:], psum_g[:], AF.Identity)
                s_tiles.append(s_g)
            s0, s1 = s_tiles

            # forward: running elementwise min per batch (quad-packed)
            nc.vector.tensor_tensor(accA[:], accA[:], s0[:], op=AluOpType.min)
            nc.vector.tensor_tensor(accB[:], accB[:], s1[:], op=AluOpType.min)

            # backward: min over the 8 batch slabs
            rr = t_pool.tile([128, 2048], dt.float16, tag="rr")
            nc.vector.tensor_tensor(rr[:], s0[:], s1[:], op=AluOpType.min)
            r2 = t_pool.tile([128, 1024], dt.float16, tag="r2")
            nc.vector.tensor_tensor(r2[:], rr[:, 0:1024], rr[:, 1024:2048],
                                    op=AluOpType.min)
            bmin = t_pool.tile([128, 512], dt.float16, tag="bmin")
            nc.vector.tensor_tensor(bmin[:], r2[:, 0:512], r2[:, 512:1024],
                                    op=AluOpType.min)
            scr = t_pool.tile([128, 512], dt.float16, tag="scr")
            nc.scalar.activation(scr[:], bmin[:], AF.Sqrt, bias=bias256[:],
                                 accum_out=bwd_sb[:, nt_:nt_ + 1])

        # --- forward finalize ---
        for g, acc in ((0, accA), (1, accB)):
            for bl in range(4):
                b = g * 4 + bl
                pt = psum_pool.tile([128, 512], dt.float16, tag="psum")
                for i in range(4):
                    nc.tensor.transpose(
                        pt[:, i * 128:(i + 1) * 128],
                        acc[:, bl * 512 + i * 128:bl * 512 + (i + 1) * 128],
                        identity[:],
                    )
                for i in range(4):
                    nc.vector.tensor_reduce(
                        fwd_mins[:, b * 4 + i:b * 4 + i + 1],
                        pt[:, i * 128:(i + 1) * 128],
                        axis=AX.X, op=AluOpType.min,
                    )

        fwd_sq = acc_pool.tile([128, NT], dt.float32, tag="fwd_sq")
        nc.vector.tensor_tensor(fwd_sq[:], fwd_mins[:], y2arr_sb[:],
                                op=AluOpType.add)
        fwd_scr = acc_pool.tile([128, NT], dt.float32, tag="fwd_scr")
        fwd_ps = acc_pool.tile([128, 1], dt.float32, tag="fwd_ps")
        nc.scalar.activation(fwd_scr[:], fwd_sq[:], AF.Sqrt, bias=bias256[:],
                             accum_out=fwd_ps[:])

        nc.sync.dma_start(bwd_d[:, :], bwd_sb[:])
        nc.sync.dma_start(fwd_d[:, :], fwd_ps[:])

    _split_waits(nc, mybir)
    return nc


def _split_waits(nc, mybir):
    """Walrus on this toolchain accepts only one sync-wait per instruction.
    Hoist all but the last wait of any instruction onto same-engine NoOps
    inserted immediately before it (engine stalls there first — semantics
    are identical)."""
    for func in nc.m.functions:
        for blk in func.blocks:
            new = []
            changed = False
            for ins in blk.instructions:
                si = ins.sync_info
                waits = list(si.on_wait) if si is not None and si.on_wait else []
                if len(waits) > 1:
                    changed = True
                    for w in waits[:-1]:
                        new.append(mybir.InstNoOp(
                            name=nc.get_next_instruction_name(),
                            engine=ins.engine,
                            ins=[], outs=[],
                            sync_info=mybir.SyncInfo(on_wait=[w], on_update=[]),
                        ))
                    ins.sync_info = mybir.SyncInfo(
                        on_wait=[waits[-1]], on_update=list(si.on_update))
                new.append(ins)
            if changed:
                blk.instructions = new


def _host_prep(x, y):
    """Build per-core input maps. x:(B,N,D) y:(B,M,D) fp32."""
    x = np.ascontiguousarray(x, np.float32)
    y = np.ascontiguousarray(y, np.float32)

    # xtp[d, nt*1024 + b*128 + lane] = x[b, nt*128+lane, d]
    xr = x.reshape(B, NT, 128, D)
    xtp = np.ascontiguousarray(
        xr.transpose(3, 1, 0, 2).reshape(D, B * N)).astype(np.float16)

    x2 = (x.astype(np.float64) ** 2).sum(-1)            # (B, N)
    x2s = (x2 - SHIFT).reshape(B, NT, 128).transpose(1, 0, 2).reshape(1, B * N)
    auglhs = np.concatenate(
        [x2s, np.ones_like(x2s)], axis=0).astype(np.float16)

    in_maps = []
    for c in range(NCORES):
        ysl = y[:, c * MC:(c + 1) * MC, :]              # (B, MC, D)
        yts = np.ascontiguousarray(
            (-2.0 * ysl).transpose(2, 0, 1).reshape(D, B * MC)
        ).astype(np.float16)
        y2 = (ysl.astype(np.float64) ** 2).sum(-1)      # (B, MC)
        y2s = (y2 - SHIFT).reshape(1, B * MC)
        augrhs = np.concatenate(
            [np.ones_like(y2s), y2s], axis=0).astype(np.float16)
        # y2arr[p, b*4+i] = y2[b, i*128+p] - 128
        y2arr = np.ascontiguousarray(
            (y2 - SHIFT).reshape(B, 4, 128).transpose(2, 0, 1).reshape(128, B * 4)
        ).astype(np.float32)
        in_maps.append({
            "xtp": xtp,
            "auglhs": auglhs,
            "yts": yts,
            "augrhs": augrhs,
            "y2arr": y2arr,
        })
    return in_maps


def _run(in_maps, trace=False, tmpdir=None):
    from concourse.bass_utils import run_bass_kernel_spmd

    if "nc" not in _cached:
        _cached["nc"] = _build_nc()
    return run_bass_kernel_spmd(
        _cached["nc"], in_maps, core_ids=list(range(NCORES)),
        trace=trace, tmpdir=tmpdir,
    )


def kernel(predicted_set, target_set):
    in_maps = _host_prep(predicted_set, target_set)
    res = _run(in_maps)
    fwd_total = np.float64(0.0)
    bwd_total = np.float64(0.0)
    for out in res.results:
        fwd_total += out["fwd_psum"].astype(np.float64).sum()
        bwd_total += out["bwd_sums"].astype(np.float64).sum()
    loss = fwd_total / (B * M) + bwd_total / (N * M)
    return np.float32(loss)
